# revision 9
# baseline (speedup 1.0000x reference)
"""Trainium2 Bass kernel for nn_CrossAttentionFusion (B=16384, D=2048, fp32).

Math: in the reference, softmax is taken over a length-1 axis, so it is
identically 1.0 and the q/k projections are dead code:

    out = (emb_b @ Wv.T + bv + emb_a) @ Wo.T + bo
        = emb_b @ (Wv.T @ Wo.T) + emb_a @ Wo.T + (Wo @ bv + bo)

The kernel computes the fused two-matmul form.  Host precomputes
Wc = Wv.T @ Wo.T (fp32) and bias_c = Wo @ bv + bo; both output-feature
contributions accumulate in PSUM on-chip.

Sharding: data-parallel over the batch dim, 2048 rows per NeuronCore.
Layout is feature-major on-device (features on partitions, rows on the
free dim), so no transposes are ever needed on-device; the host
transposes the embedding shards in and the output shards back out.

Numerics: matmul operands are cast to bf16 on host (PE array runs bf16
at 1 cycle/row vs 4 for fp32); accumulation is fp32 in PSUM.
"""

import numpy as np
import ml_dtypes

import concourse.bass as bass
import concourse.mybir as mybir
import concourse.tile as tile
from concourse import bacc
from concourse.bass import ts
from concourse.bass_utils import run_bass_kernel_spmd

BF16 = ml_dtypes.bfloat16

NCORES = 8
B = 16384
D = 2048
R = B // NCORES          # rows per core
P = 128                  # partitions
KO = D // P              # contraction chunks (16)
MO = D // P              # output-feature chunks (16)
NT = 512                 # rows per matmul (moving free dim)
NB = R // NT             # row blocks per core (4)

_NC_CACHE = {}

# Exposed for test harnesses: BassKernelResults of the most recent run.
LAST_RESULT = None


def _build_bass(D=D, R=R, NT=NT):
    """Per-core program: outt[D, R] = Wc.T-path(ebt) + WoT-path(eat) + bias."""
    KO = D // P
    MO = D // P
    NB = R // NT
    # Bacc (not raw Bass): its compile() splits multi-sem waits into
    # InstEventSemaphore (TRN2 allows at most one sync wait per instruction).
    nc = bacc.Bacc(None, target_bir_lowering=False)
    f32 = mybir.dt.float32
    bf16 = mybir.dt.bfloat16

    ebt_d = nc.dram_tensor("ebt", [D, R], bf16, kind="ExternalInput")
    eat_d = nc.dram_tensor("eat", [D, R], bf16, kind="ExternalInput")
    wc_d = nc.dram_tensor("wc", [D, D], bf16, kind="ExternalInput")
    wot_d = nc.dram_tensor("wot", [D, D], bf16, kind="ExternalInput")
    bias_d = nc.dram_tensor("bias", [D], f32, kind="ExternalInput")
    outt_d = nc.dram_tensor("outt", [D, R], f32, kind="ExternalOutput")

    ebt_r = ebt_d.rearrange("(ko p) r -> p ko r", p=P)
    eat_r = eat_d.rearrange("(ko p) r -> p ko r", p=P)
    wc_r = wc_d.rearrange("(ko p) m -> p ko m", p=P)
    wot_r = wot_d.rearrange("(ko p) m -> p ko m", p=P)
    bias_r = bias_d.rearrange("(mo p) -> p mo", p=P)

    with tile.TileContext(nc) as tc:
        with (
            tc.tile_pool(name="weights", bufs=1) as wpool,
            tc.tile_pool(name="acts", bufs=2) as apool,
            tc.tile_pool(name="outs", bufs=4) as opool,
            tc.tile_pool(name="psum", bufs=8, space="PSUM") as pspool,
        ):
            wc_sb = wpool.tile([P, KO, D], bf16, tag="wc")
            wot_sb = wpool.tile([P, KO, D], bf16, tag="wot")
            bias_st = wpool.tile([P, MO], f32, tag="bias_st")
            bias_sb = wpool.tile([P, MO], f32, tag="bias")

            # Stage bias through a DVE copy: the per-tile bias-add TensorTensor
            # then depends only on PE (TT has a single HW sync-wait slot).
            nc.sync.dma_start(bias_st[:], bias_r[:])
            nc.vector.tensor_copy(bias_sb[:], bias_st[:])

            # Row-block schedule: small leading blocks so early compute gates
            # on few bytes; steady state uses full 512-wide blocks.
            widths = (256, 256, 512, 512, 512) if R == 2048 else (NT,) * (R // NT)
            row_blocks = []
            r0 = 0
            for w in widths:
                row_blocks.append((r0, w))
                r0 += w
            assert r0 == R

            def load_acts(r0, w):
                eb_t = apool.tile([P, KO, NT], bf16, tag="eb")
                ea_t = apool.tile([P, KO, NT], bf16, tag="ea")
                for ko in range(KO):
                    nc.sync.dma_start(eb_t[:, ko, :w], ebt_r[:, ko, r0 : r0 + w])
                    nc.sync.dma_start(ea_t[:, ko, :w], eat_r[:, ko, r0 : r0 + w])
                return eb_t, ea_t

            # Weight warmup: first output-column block (mo=0) as per-ko 32KB
            # DMAs — the first LDWEIGHTS only gates on one of these.
            for ko in range(KO):
                nc.sync.dma_start(wc_sb[:, ko, 0:P], wc_r[:, ko, 0:P])
                nc.sync.dma_start(wot_sb[:, ko, 0:P], wot_r[:, ko, 0:P])
            # Acts for the first two row blocks, ahead of the weight remainder.
            acts_pre = [load_acts(*row_blocks[0]), load_acts(*row_blocks[1])]
            # Remaining weight columns in 256-wide blocks (then a 128 tail).
            for off in range(P, D, 2 * P):
                w = min(2 * P, D - off)
                nc.sync.dma_start(wc_sb[:, :, off : off + w], wc_r[:, :, off : off + w])
                nc.sync.dma_start(
                    wot_sb[:, :, off : off + w], wot_r[:, :, off : off + w]
                )

            for nb, (r0, w) in enumerate(row_blocks):
                eb_t, ea_t = acts_pre[nb] if nb < 2 else load_acts(r0, w)

                for mo in range(MO):
                    ps = pspool.tile([P, NT], f32, tag="ps")
                    for ko in range(KO):
                        nc.tensor.matmul(
                            ps[:, :w],
                            wc_sb[:, ko, ts(mo, P)],
                            eb_t[:, ko, :w],
                            start=(ko == 0),
                            stop=False,
                        )
                    for ko in range(KO):
                        nc.tensor.matmul(
                            ps[:, :w],
                            wot_sb[:, ko, ts(mo, P)],
                            ea_t[:, ko, :w],
                            start=False,
                            stop=(ko == KO - 1),
                        )
                    ot = opool.tile([P, NT], f32, tag="ot")
                    nc.vector.tensor_tensor(
                        ot[:, :w],
                        ps[:, :w],
                        bias_sb[:, mo : mo + 1].to_broadcast((P, w)),
                        mybir.AluOpType.add,
                    )
                    nc.sync.dma_start(outt_d[ts(mo, P), r0 : r0 + w], ot[:, :w])

    nc.compile()
    return nc


def kernel(emb_a, emb_b, Wq, bq, Wk, bk, Wv, bv, Wo, bo):
    global LAST_RESULT
    emb_a = np.asarray(emb_a, dtype=np.float32)
    emb_b = np.asarray(emb_b, dtype=np.float32)
    Wv = np.asarray(Wv, dtype=np.float32)
    bv = np.asarray(bv, dtype=np.float32)
    Wo = np.asarray(Wo, dtype=np.float32)
    bo = np.asarray(bo, dtype=np.float32)

    # Fused weights / bias (q/k are dead code: softmax over a length-1
    # axis is exactly 1.0).
    Wc = np.matmul(Wv.T, Wo.T)                       # [D_in, D_out] fp32
    bias = (Wo.astype(np.float64) @ bv.astype(np.float64) + bo).astype(np.float32)

    wc_bf = Wc.astype(BF16)
    wot_bf = Wo.T.astype(BF16, order="C")

    ea_bf = emb_a.astype(BF16)
    eb_bf = emb_b.astype(BF16)

    in_maps = []
    for c in range(NCORES):
        sl = slice(c * R, (c + 1) * R)
        in_maps.append(
            {
                "ebt": np.ascontiguousarray(eb_bf[sl].T),
                "eat": np.ascontiguousarray(ea_bf[sl].T),
                "wc": wc_bf,
                "wot": wot_bf,
                "bias": bias,
            }
        )

    if "nc" not in _NC_CACHE:
        _NC_CACHE["nc"] = _build_bass()
    nc = _NC_CACHE["nc"]

    res = run_bass_kernel_spmd(nc, in_maps, core_ids=list(range(NCORES)))
    LAST_RESULT = res

    out = np.empty((B, D), dtype=np.float32)
    for c in range(NCORES):
        out[c * R : (c + 1) * R, :] = res.results[c]["outt"].T
    return out


# revision 10
# speedup vs baseline: 1.1049x; 1.1049x over previous
"""Trainium2 Bass kernel for nn_CrossAttentionFusion (B=16384, D=2048, fp32).

Math: in the reference, softmax is taken over a length-1 axis, so it is
identically 1.0 and the q/k projections are dead code:

    out = (emb_b @ Wv.T + bv + emb_a) @ Wo.T + bo
        = emb_b @ (Wv.T @ Wo.T) + emb_a @ Wo.T + (Wo @ bv + bo)

The kernel computes the fused two-matmul form.  Host precomputes
Wc = Wv.T @ Wo.T (fp32) and bias_c = Wo @ bv + bo; both output-feature
contributions accumulate in PSUM on-chip.

Sharding: data-parallel over the batch dim, 2048 rows per NeuronCore.
Layout is feature-major on-device (features on partitions, rows on the
free dim), so no transposes are ever needed on-device; the host
transposes the embedding shards in and the output shards back out.

Numerics: matmul operands are cast to bf16 on host (PE array runs bf16
at 1 cycle/row vs 4 for fp32); accumulation is fp32 in PSUM.
"""

import numpy as np
import ml_dtypes

import concourse.bass as bass
import concourse.mybir as mybir
import concourse.tile as tile
from concourse import bacc
from concourse.bass import ts
from concourse.bass_utils import run_bass_kernel_spmd

BF16 = ml_dtypes.bfloat16

NCORES = 8
B = 16384
D = 2048
R = B // NCORES          # rows per core
P = 128                  # partitions
KO = D // P              # contraction chunks (16)
MO = D // P              # output-feature chunks (16)
NT = 512                 # rows per matmul (moving free dim)
NB = R // NT             # row blocks per core (4)

_NC_CACHE = {}

# Exposed for test harnesses: BassKernelResults of the most recent run.
LAST_RESULT = None


def _build_bass(D=D, R=R, NT=NT):
    """Per-core program: outt[D, R] = Wc.T-path(ebt) + WoT-path(eat) + bias."""
    KO = D // P
    MO = D // P
    NB = R // NT
    # Bacc (not raw Bass): its compile() splits multi-sem waits into
    # InstEventSemaphore (TRN2 allows at most one sync wait per instruction).
    nc = bacc.Bacc(None, target_bir_lowering=False)
    f32 = mybir.dt.float32
    bf16 = mybir.dt.bfloat16

    ebt_d = nc.dram_tensor("ebt", [D, R], bf16, kind="ExternalInput")
    eat_d = nc.dram_tensor("eat", [D, R], bf16, kind="ExternalInput")
    wc_d = nc.dram_tensor("wc", [D, D], bf16, kind="ExternalInput")
    wot_d = nc.dram_tensor("wot", [D, D], bf16, kind="ExternalInput")
    bias_d = nc.dram_tensor("bias", [D], f32, kind="ExternalInput")
    outt_d = nc.dram_tensor("outt", [D, R], f32, kind="ExternalOutput")

    ebt_r = ebt_d.rearrange("(ko p) r -> p ko r", p=P)
    eat_r = eat_d.rearrange("(ko p) r -> p ko r", p=P)
    wc_r = wc_d.rearrange("(ko p) m -> p ko m", p=P)
    wot_r = wot_d.rearrange("(ko p) m -> p ko m", p=P)
    bias_r = bias_d.rearrange("(mo p) -> p mo", p=P)

    with tile.TileContext(nc) as tc:
        with (
            tc.tile_pool(name="weights", bufs=1) as wpool,
            tc.tile_pool(name="acts", bufs=2) as apool,
            tc.tile_pool(name="outs", bufs=4) as opool,
            tc.tile_pool(name="psum", bufs=8, space="PSUM") as pspool,
        ):
            wc_sb = wpool.tile([P, KO, D], bf16, tag="wc")
            wot_sb = wpool.tile([P, KO, D], bf16, tag="wot")
            bias_st = wpool.tile([P, MO], f32, tag="bias_st")
            bias_sb = wpool.tile([P, MO], f32, tag="bias")

            # Stage bias through a DVE copy: the per-tile bias-add TensorTensor
            # then depends only on PE (TT has a single HW sync-wait slot).
            nc.sync.dma_start(bias_st[:], bias_r[:])
            nc.vector.tensor_copy(bias_sb[:], bias_st[:])

            row_blocks = [(i * NT, NT) for i in range(R // NT)]

            # Activations ride the ACT HWDGE ring (nc.scalar) so they never
            # queue behind the 16MB weight stream on the SP ring (nc.sync).
            def load_acts(r0, w):
                eb_t = apool.tile([P, KO, NT], bf16, tag="eb")
                ea_t = apool.tile([P, KO, NT], bf16, tag="ea")
                for ko in range(KO):
                    nc.scalar.dma_start(eb_t[:, ko, :w], ebt_r[:, ko, r0 : r0 + w])
                    nc.scalar.dma_start(ea_t[:, ko, :w], eat_r[:, ko, r0 : r0 + w])
                return eb_t, ea_t

            # First weight column-block (mo=0..1) split per-ko: the first
            # LDWEIGHTS gates on a single 64KB DMA instead of a 1MB block.
            MBW = 2 * P
            for ko in range(KO):
                nc.sync.dma_start(wc_sb[:, ko, 0:MBW], wc_r[:, ko, 0:MBW])
                nc.sync.dma_start(wot_sb[:, ko, 0:MBW], wot_r[:, ko, 0:MBW])
            acts_pre = load_acts(*row_blocks[0])
            for off in range(MBW, D, MBW):
                nc.sync.dma_start(
                    wc_sb[:, :, off : off + MBW], wc_r[:, :, off : off + MBW]
                )
                nc.sync.dma_start(
                    wot_sb[:, :, off : off + MBW], wot_r[:, :, off : off + MBW]
                )

            for nb, (r0, w) in enumerate(row_blocks):
                eb_t, ea_t = acts_pre if nb == 0 else load_acts(r0, w)

                for mo in range(MO):
                    ps = pspool.tile([P, NT], f32, tag="ps")
                    for ko in range(KO):
                        nc.tensor.matmul(
                            ps[:, :w],
                            wc_sb[:, ko, ts(mo, P)],
                            eb_t[:, ko, :w],
                            start=(ko == 0),
                            stop=False,
                        )
                    for ko in range(KO):
                        nc.tensor.matmul(
                            ps[:, :w],
                            wot_sb[:, ko, ts(mo, P)],
                            ea_t[:, ko, :w],
                            start=False,
                            stop=(ko == KO - 1),
                        )
                    ot = opool.tile([P, NT], f32, tag="ot")
                    nc.vector.tensor_tensor(
                        ot[:, :w],
                        ps[:, :w],
                        bias_sb[:, mo : mo + 1].to_broadcast((P, w)),
                        mybir.AluOpType.add,
                    )
                    nc.sync.dma_start(outt_d[ts(mo, P), r0 : r0 + w], ot[:, :w])

    nc.compile()
    return nc


def kernel(emb_a, emb_b, Wq, bq, Wk, bk, Wv, bv, Wo, bo):
    global LAST_RESULT
    emb_a = np.asarray(emb_a, dtype=np.float32)
    emb_b = np.asarray(emb_b, dtype=np.float32)
    Wv = np.asarray(Wv, dtype=np.float32)
    bv = np.asarray(bv, dtype=np.float32)
    Wo = np.asarray(Wo, dtype=np.float32)
    bo = np.asarray(bo, dtype=np.float32)

    # Fused weights / bias (q/k are dead code: softmax over a length-1
    # axis is exactly 1.0).
    Wc = np.matmul(Wv.T, Wo.T)                       # [D_in, D_out] fp32
    bias = (Wo.astype(np.float64) @ bv.astype(np.float64) + bo).astype(np.float32)

    wc_bf = Wc.astype(BF16)
    wot_bf = Wo.T.astype(BF16, order="C")

    ea_bf = emb_a.astype(BF16)
    eb_bf = emb_b.astype(BF16)

    in_maps = []
    for c in range(NCORES):
        sl = slice(c * R, (c + 1) * R)
        in_maps.append(
            {
                "ebt": np.ascontiguousarray(eb_bf[sl].T),
                "eat": np.ascontiguousarray(ea_bf[sl].T),
                "wc": wc_bf,
                "wot": wot_bf,
                "bias": bias,
            }
        )

    if "nc" not in _NC_CACHE:
        _NC_CACHE["nc"] = _build_bass()
    nc = _NC_CACHE["nc"]

    res = run_bass_kernel_spmd(nc, in_maps, core_ids=list(range(NCORES)))
    LAST_RESULT = res

    out = np.empty((B, D), dtype=np.float32)
    for c in range(NCORES):
        out[c * R : (c + 1) * R, :] = res.results[c]["outt"].T
    return out


# revision 11
# speedup vs baseline: 1.1056x; 1.0006x over previous
"""Trainium2 Bass kernel for nn_CrossAttentionFusion (B=16384, D=2048, fp32).

Math: in the reference, softmax is taken over a length-1 axis, so it is
identically 1.0 and the q/k projections are dead code:

    out = (emb_b @ Wv.T + bv + emb_a) @ Wo.T + bo
        = emb_b @ (Wv.T @ Wo.T) + emb_a @ Wo.T + (Wo @ bv + bo)

The kernel computes the fused two-matmul form.  Host precomputes
Wc = Wv.T @ Wo.T (fp32) and bias_c = Wo @ bv + bo; both output-feature
contributions accumulate in PSUM on-chip.

Sharding: data-parallel over the batch dim, 2048 rows per NeuronCore.
Layout is feature-major on-device (features on partitions, rows on the
free dim), so no transposes are ever needed on-device; the host
transposes the embedding shards in and the output shards back out.

Numerics: matmul operands are cast to bf16 on host (PE array runs bf16
at 1 cycle/row vs 4 for fp32); accumulation is fp32 in PSUM.
"""

import numpy as np
import ml_dtypes

import concourse.bass as bass
import concourse.mybir as mybir
import concourse.tile as tile
from concourse import bacc
from concourse.bass import ts
from concourse.bass_utils import run_bass_kernel_spmd

BF16 = ml_dtypes.bfloat16

NCORES = 8
B = 16384
D = 2048
R = B // NCORES          # rows per core
P = 128                  # partitions
KO = D // P              # contraction chunks (16)
MO = D // P              # output-feature chunks (16)
NT = 512                 # rows per matmul (moving free dim)
NB = R // NT             # row blocks per core (4)

_NC_CACHE = {}

# Exposed for test harnesses: BassKernelResults of the most recent run.
LAST_RESULT = None


def _build_bass(D=D, R=R, NT=NT):
    """Per-core program: outt[D, R] = Wc.T-path(ebt) + WoT-path(eat) + bias."""
    KO = D // P
    MO = D // P
    NB = R // NT
    # Bacc (not raw Bass): its compile() splits multi-sem waits into
    # InstEventSemaphore (TRN2 allows at most one sync wait per instruction).
    nc = bacc.Bacc(None, target_bir_lowering=False)
    f32 = mybir.dt.float32
    bf16 = mybir.dt.bfloat16

    ebt_d = nc.dram_tensor("ebt", [D, R], bf16, kind="ExternalInput")
    eat_d = nc.dram_tensor("eat", [D, R], bf16, kind="ExternalInput")
    wc_d = nc.dram_tensor("wc", [D, D], bf16, kind="ExternalInput")
    wot_d = nc.dram_tensor("wot", [D, D], bf16, kind="ExternalInput")
    bias_d = nc.dram_tensor("bias", [D], f32, kind="ExternalInput")
    outt_d = nc.dram_tensor("outt", [D, R], f32, kind="ExternalOutput")

    ebt_r = ebt_d.rearrange("(ko p) r -> p ko r", p=P)
    eat_r = eat_d.rearrange("(ko p) r -> p ko r", p=P)
    wc_r = wc_d.rearrange("(ko p) m -> p ko m", p=P)
    wot_r = wot_d.rearrange("(ko p) m -> p ko m", p=P)
    bias_r = bias_d.rearrange("(mo p) -> p mo", p=P)

    with tile.TileContext(nc) as tc:
        with (
            tc.tile_pool(name="weights", bufs=1) as wpool,
            tc.tile_pool(name="acts", bufs=2) as apool,
            tc.tile_pool(name="outs", bufs=4) as opool,
            tc.tile_pool(name="psum", bufs=8, space="PSUM") as pspool,
        ):
            wc_sb = wpool.tile([P, KO, D], bf16, tag="wc")
            wot_sb = wpool.tile([P, KO, D], bf16, tag="wot")
            bias_st = wpool.tile([P, MO], f32, tag="bias_st")
            bias_sb = wpool.tile([P, MO], f32, tag="bias")

            # PE warm-up: dummy matmuls on memset tiles while the first DMAs
            # land. ~3.4us of sustained PE activity flips the HAM clock gate
            # to 2.4GHz before real work arrives. Shares the "ps" psum tag so
            # no extra PSUM bank is needed; result is never read.
            wu_w = wpool.tile([P, P], bf16, tag="wu_w")
            wu_x = wpool.tile([P, NT], bf16, tag="wu_x")
            nc.vector.memset(wu_w[:], 0.0)
            nc.vector.memset(wu_x[:], 0.0)
            wu_ps = pspool.tile([P, NT], f32, tag="ps")
            for i in range(16):
                nc.tensor.matmul(
                    wu_ps[:], wu_w[:], wu_x[:], start=(i == 0), stop=(i == 15)
                )

            # Stage bias through a DVE copy: the per-tile bias-add TensorTensor
            # then depends only on PE (TT has a single HW sync-wait slot).
            nc.sync.dma_start(bias_st[:], bias_r[:])
            nc.vector.tensor_copy(bias_sb[:], bias_st[:])

            row_blocks = [(i * NT, NT) for i in range(R // NT)]

            # Activations ride the ACT HWDGE ring (nc.scalar) so they never
            # queue behind the 16MB weight stream on the SP ring (nc.sync).
            def load_acts(r0, w):
                eb_t = apool.tile([P, KO, NT], bf16, tag="eb")
                ea_t = apool.tile([P, KO, NT], bf16, tag="ea")
                for ko in range(KO):
                    nc.scalar.dma_start(eb_t[:, ko, :w], ebt_r[:, ko, r0 : r0 + w])
                    nc.scalar.dma_start(ea_t[:, ko, :w], eat_r[:, ko, r0 : r0 + w])
                return eb_t, ea_t

            # First weight column-block (mo=0..1) split per-ko: the first
            # LDWEIGHTS gates on a single 64KB DMA instead of a 1MB block.
            MBW = 2 * P
            for ko in range(KO):
                nc.sync.dma_start(wc_sb[:, ko, 0:MBW], wc_r[:, ko, 0:MBW])
                nc.sync.dma_start(wot_sb[:, ko, 0:MBW], wot_r[:, ko, 0:MBW])
            acts_pre = load_acts(*row_blocks[0])
            for off in range(MBW, D, MBW):
                nc.sync.dma_start(
                    wc_sb[:, :, off : off + MBW], wc_r[:, :, off : off + MBW]
                )
                nc.sync.dma_start(
                    wot_sb[:, :, off : off + MBW], wot_r[:, :, off : off + MBW]
                )

            for nb, (r0, w) in enumerate(row_blocks):
                eb_t, ea_t = acts_pre if nb == 0 else load_acts(r0, w)

                for mo in range(MO):
                    ps = pspool.tile([P, NT], f32, tag="ps")
                    for ko in range(KO):
                        nc.tensor.matmul(
                            ps[:, :w],
                            wc_sb[:, ko, ts(mo, P)],
                            eb_t[:, ko, :w],
                            start=(ko == 0),
                            stop=False,
                        )
                    for ko in range(KO):
                        nc.tensor.matmul(
                            ps[:, :w],
                            wot_sb[:, ko, ts(mo, P)],
                            ea_t[:, ko, :w],
                            start=False,
                            stop=(ko == KO - 1),
                        )
                    ot = opool.tile([P, NT], f32, tag="ot")
                    nc.vector.tensor_tensor(
                        ot[:, :w],
                        ps[:, :w],
                        bias_sb[:, mo : mo + 1].to_broadcast((P, w)),
                        mybir.AluOpType.add,
                    )
                    nc.sync.dma_start(outt_d[ts(mo, P), r0 : r0 + w], ot[:, :w])

    nc.compile()
    return nc


def kernel(emb_a, emb_b, Wq, bq, Wk, bk, Wv, bv, Wo, bo):
    global LAST_RESULT
    emb_a = np.asarray(emb_a, dtype=np.float32)
    emb_b = np.asarray(emb_b, dtype=np.float32)
    Wv = np.asarray(Wv, dtype=np.float32)
    bv = np.asarray(bv, dtype=np.float32)
    Wo = np.asarray(Wo, dtype=np.float32)
    bo = np.asarray(bo, dtype=np.float32)

    # Fused weights / bias (q/k are dead code: softmax over a length-1
    # axis is exactly 1.0).
    Wc = np.matmul(Wv.T, Wo.T)                       # [D_in, D_out] fp32
    bias = (Wo.astype(np.float64) @ bv.astype(np.float64) + bo).astype(np.float32)

    wc_bf = Wc.astype(BF16)
    wot_bf = Wo.T.astype(BF16, order="C")

    ea_bf = emb_a.astype(BF16)
    eb_bf = emb_b.astype(BF16)

    in_maps = []
    for c in range(NCORES):
        sl = slice(c * R, (c + 1) * R)
        in_maps.append(
            {
                "ebt": np.ascontiguousarray(eb_bf[sl].T),
                "eat": np.ascontiguousarray(ea_bf[sl].T),
                "wc": wc_bf,
                "wot": wot_bf,
                "bias": bias,
            }
        )

    if "nc" not in _NC_CACHE:
        _NC_CACHE["nc"] = _build_bass()
    nc = _NC_CACHE["nc"]

    res = run_bass_kernel_spmd(nc, in_maps, core_ids=list(range(NCORES)))
    LAST_RESULT = res

    out = np.empty((B, D), dtype=np.float32)
    for c in range(NCORES):
        out[c * R : (c + 1) * R, :] = res.results[c]["outt"].T
    return out
